# revision 76
# baseline (speedup 1.0000x reference)
"""EvaLinearAttention Trainium2 kernel.

Per-core math (one batch element per core, 8 cores, SPMD):
  qkv = x @ W_qkv.T + bias ; RoPE on q,k (interleaved pairs, prefix token 0
  identity) ; phi_* = softmax(*, -1)
  kv = phi_k.T @ v ; ksum = sum_n phi_k ; z = phi_q @ ksum
  attn = (phi_q @ kv) / (z + eps) ; out = LN(attn) @ proj_w.T + proj_b

Reformulation / schedule:
  - Phase 1 (one pass over x): token-major k/v per 128-token block with RoPE
    + exp; k-softmax denominator folded into v (vn = v/rowsum(exp_k));
    kv accumulated in bf16 at 130-free (128 kv cols + 2 ksum cols) across
    all 4096 tokens in 2 PSUM banks (3 pair-chains per bank). q computed
    feature-major per supertile, RoPE'd, exp'd (q softmax normalization
    cancels between attn numerator and z), stored bf16 in SBUF for phase 2.
  - Phase 2 (no HBM reads): per pair ya = kvext^T eq and z = ksum_bcast^T eq
    (block-diagonal broadcast ksum makes z come out already replicated to
    all 128 feature partitions, so no separate broadcast matmul); att =
    ya * 1/z. LayerNorm folded into the projection: pwg gets an extra ones
    column so sum_c(att) rides along in the proj PSUM; variance from a
    Square+ones-matmul chain; out = r*P + (r*mu)*(-wsum) + const applied
    as two fused scalar_tensor_tensor ops. Exp and Sqrt live in different
    activation tables; phase separation means 2 table loads total.
"""

import functools
import sys

import numpy as np

sys.path.insert(0, "/opt/trn_rl_repo")

import concourse.bass as bass
import concourse.bacc as bacc
import concourse.mybir as mybir
import concourse.tile as tile
from concourse.bass_utils import run_bass_kernel_spmd

B, N, C, H, D = 8, 4096, 768, 12, 64
NPT = 1
ST = 512            # tokens per super-tile
NST = N // ST       # 8
NB = ST // 128      # 4 sub-tiles of 128 tokens
NCT = C // 128      # 6 contraction tiles
NPAIR = H // 2      # 6 head pairs
EPS_LN = 1e-5
FP = mybir.dt.float32
FR = mybir.dt.float32r
BF = mybir.dt.bfloat16

PAIRSWAP32 = [i ^ 1 for i in range(32)]


def _mm(nc, out, lhsT, rhs, start, stop):
    nc.tensor.matmul(out, lhsT.bitcast(FR), rhs.bitcast(FR), start=start,
                     stop=stop)


def _mmb(nc, out, lhsT, rhs, start, stop):
    nc.tensor.matmul(out, lhsT, rhs, start=start, stop=stop)


def build_nc(vb_zero=True):
    nc = bacc.Bacc("TRN2", target_bir_lowering=False)

    xpk = nc.dram_tensor("xpk", [NST, 128, NCT * ST], FR, kind="ExternalInput")
    wqT = nc.dram_tensor("wqT", [C, C], FR, kind="ExternalInput")
    wkvT = nc.dram_tensor("wkvT", [C, 2 * C], FR, kind="ExternalInput")
    cosT2 = nc.dram_tensor("cosT2", [128, N], FP, kind="ExternalInput")
    sinT2 = nc.dram_tensor("sinT2", [128, N], FP, kind="ExternalInput")
    rope_pk = nc.dram_tensor("rope_pk", [NST, 128, NB, 2, D], FP,
                             kind="ExternalInput")
    qb = nc.dram_tensor("qb", [C], FP, kind="ExternalInput")
    vb = nc.dram_tensor("vb", [C], FP, kind="ExternalInput")
    pwgT = nc.dram_tensor("pwgT", [C, 770], FR, kind="ExternalInput")
    wsn2 = nc.dram_tensor("wsn2", [C], FP, kind="ExternalInput")
    constb = nc.dram_tensor("constb", [C], FP, kind="ExternalInput")
    out = nc.dram_tensor("out", [N, C], FP, kind="ExternalOutput")

    with tile.TileContext(nc) as tc:
        with (
            tc.tile_pool(name="common", bufs=1) as common,
            tc.tile_pool(name="xa", bufs=2) as xpool,
        ):
            # persistent tiles
            wqs = [common.tile([128, C], FR, name=f"wq{ct}", tag=f"wq{ct}")
                   for ct in range(NCT)]
            pwgs = [common.tile([128, 770], FR, name=f"pwg{ct}",
                                tag=f"pwg{ct}") for ct in range(NCT)]
            vbb = common.tile([128, C], FP)
            qb_sb = common.tile([128, NCT], FP)
            cb_sb = common.tile([128, C], FP)
            wsb = common.tile([128, C], FP)
            one11 = common.tile([1, 1], FP)
            nc.vector.memset(one11, 1.0)
            # bf16 constants via f32 memset + converting copy (bf16 memset
            # fails the hw ISA check)
            onesf = common.tile([128, 1], FP)
            nc.vector.memset(onesf, 1.0)
            ones128b = common.tile([128, 1], BF)
            nc.vector.tensor_copy(ones128b, onesf)
            zf = common.tile([128, 3, 128], FP)
            nc.vector.memset(zf, 0.0)
            # per-bank tiles so phase 2's first matmuls only wait on their
            # own bank's assembly ops
            kvx2 = [common.tile([128, 3, 128], BF, name=f"kvx{i}",
                                tag=f"kvx{i}") for i in range(2)]
            ksdb = [common.tile([128, 3, 128], BF, name=f"ksd{i}",
                                tag=f"ksd{i}") for i in range(2)]
            for i in range(2):
                nc.vector.tensor_copy(kvx2[i], zf)
                nc.vector.tensor_copy(ksdb[i], zf)
            eqs = [common.tile([128, N], BF, name=f"eqs{fq}", tag=f"eqs{fq}")
                   for fq in range(NPAIR)]

            # qb is tiny and needed by the first q exp; the rest are emitted
            # after the weight loads
            nc.gpsimd.dma_start(
                out=qb_sb, in_=qb.rearrange("(a p) -> p a", p=128))

            # ============ PHASE 1: x -> (kv, ksum) accum + eq store ========
            with (
                tc.tile_pool(name="wkvp", bufs=1) as wkvp,
                tc.tile_pool(name="sa", bufs=2) as sa,
                tc.tile_pool(name="wk", bufs=2) as wk,
                tc.tile_pool(name="ekp", bufs=2) as ekpool,
                tc.tile_pool(name="vnxp", bufs=2) as vnxpool,
                tc.tile_pool(name="qw", bufs=2) as qw,
                tc.tile_pool(name="kvac", bufs=1, space="PSUM") as kvacp,
                tc.tile_pool(name="kvo", bufs=4, space="PSUM") as kvop,
                tc.tile_pool(name="qps", bufs=2, space="PSUM") as qpool,
            ):
                wkvs = [wkvp.tile([128, 2 * C], FR, name=f"wkv{ct}",
                                  tag=f"wkv{ct}") for ct in range(NCT)]
                dmaeng = [nc.gpsimd, nc.sync, nc.scalar]
                # st0 x halves go out first so the first q chain (which only
                # needs wq + x) can start ~2.5us in; wq interleaves with them,
                # wkv rides behind (not needed until the first k/v block at
                # ~10us), bulk broadcast consts last.
                # head DMA schedule: the q loop needs wq+x by ~2.5us and the
                # first k/v block needs wkv by ~10us; the Pool engine must be
                # free for q-path compute from ~5.5us, so it only issues the
                # earliest loads; everything else rides on SP/ACT queues in
                # need-order
                xa0 = xpool.tile([128, NCT, ST], FR, tag="xa")
                x0r = xpk[0].rearrange("p (a t) -> p a t", t=ST)
                nc.sync.dma_start(out=xa0[:, 0:1, :], in_=x0r[:, 0:1, :])
                nc.sync.dma_start(out=xa0[:, 1:3, :], in_=x0r[:, 1:3, :])
                nc.scalar.dma_start(out=xa0[:, 3:6, :], in_=x0r[:, 3:6, :])
                ropa0 = sa.tile([128, NB, 2, D], FP, tag="ropa")
                cT0 = sa.tile([128, ST], FP, tag="cT")
                sT0 = sa.tile([128, ST], FP, tag="sT")
                nc.gpsimd.dma_start(out=wqs[0], in_=wqT[0:128, :])
                nc.gpsimd.dma_start(out=wqs[3], in_=wqT[384:512, :])
                nc.sync.dma_start(out=wqs[1], in_=wqT[128:256, :])
                nc.sync.dma_start(out=wqs[4], in_=wqT[512:640, :])
                nc.scalar.dma_start(out=wqs[2], in_=wqT[256:384, :])
                nc.scalar.dma_start(out=wqs[5], in_=wqT[640:768, :])
                nc.scalar.dma_start(out=cT0, in_=cosT2[:, 0:ST])
                nc.scalar.dma_start(out=sT0, in_=sinT2[:, 0:ST])
                nc.gpsimd.dma_start(out=wkvs[0], in_=wkvT[0:128, :])
                nc.gpsimd.dma_start(out=wkvs[3], in_=wkvT[384:512, :])
                nc.sync.dma_start(out=wkvs[1], in_=wkvT[128:256, :])
                nc.sync.dma_start(out=wkvs[4], in_=wkvT[512:640, :])
                nc.scalar.dma_start(out=wkvs[2], in_=wkvT[256:384, :])
                nc.scalar.dma_start(out=wkvs[5], in_=wkvT[640:768, :])
                nc.scalar.dma_start(out=ropa0, in_=rope_pk[0])

                kvac = [
                    kvacp.tile([128, 512], FP, tag=f"kvac{i}", name=f"kvac{i}")
                    for i in range(2)
                ]
                pending_kv = None

                def emit_kv_half(args, half):
                    ek, vnx, first, last = args
                    for pr in range(3 * half, 3 * half + 3):
                        _mmb(
                            nc,
                            kvac[half][:, (pr % 3) * 130 : (pr % 3) * 130 + 130],
                            ek[:, pr * 128 : (pr + 1) * 128],
                            vnx[:, pr, :],
                            first and pr % 3 == 0, last,
                        )

                def emit_kv(args):
                    emit_kv_half(args, 0)
                    emit_kv_half(args, 1)

                def do_q(st, tsl, xa, cT, sT):
                    # --- q: feature-major, RoPE, exp, store bf16 ---
                    nonlocal pending_kv
                    for fq in range(NPAIR):
                        qps = qpool.tile([128, ST], FP, tag="q")
                        for ct in range(NCT):
                            _mm(
                                nc, qps,
                                wqs[ct][:, fq * 128 : (fq + 1) * 128],
                                xa[:, ct, :],
                                ct == 0, ct == NCT - 1,
                            )
                        if fq == 0 and pending_kv is not None:
                            # previous block's kv mms ride behind the first q
                            # chain so the PE isn't waiting on vnx
                            emit_kv(pending_kv)
                            pending_kv = None
                        qs = qw.tile([128, ST], FP, tag="qs")
                        nc.vector.stream_shuffle(qs, qps, PAIRSWAP32)
                        # gpsimd cannot touch PSUM on hw; qps reads stay on DVE
                        t1q = qw.tile([128, ST], FP, tag="qt1")
                        nc.vector.tensor_mul(t1q, qps, cT)
                        t2q = qw.tile([128, ST], FP, tag="qt2")
                        nc.gpsimd.tensor_mul(t2q, qs, sT)
                        eqin = qw.tile([128, ST], FP, tag="eqin")
                        nc.gpsimd.tensor_add(eqin, t1q, t2q)
                        nc.scalar.activation(
                            out=eqs[fq][:, tsl],
                            in_=eqin,
                            func=mybir.ActivationFunctionType.Exp,
                            bias=qb_sb[:, fq : fq + 1],
                        )

                def do_blocks(st, tsl, xa, ropa):
                    nonlocal pending_kv
                    for b in range(NB):
                        if pending_kv is not None:
                            emit_kv(pending_kv)
                            pending_kv = None
                        ek = ekpool.tile([128, C], BF, tag="ek")
                        # --- k: wkv cols [0, 768), 2 tiles of 384 ---
                        for kt in range(2):
                            kps = kvop.tile([128, 384], FP, tag="kv_mm")
                            for ct in range(NCT):
                                _mm(
                                    nc, kps,
                                    xa[:, ct, b * 128 : (b + 1) * 128],
                                    wkvs[ct][:, kt * 384 : (kt + 1) * 384],
                                    ct == 0, ct == NCT - 1,
                                )
                            cosb = bass.AP(
                                tensor=ropa.tensor,
                                offset=ropa.offset + (b * 2) * D,
                                ap=[ropa.ap[0], [0, 6], [1, D]],
                            )
                            # PSUM -> SBUF copy so the gpsimd rope ops can
                            # read it (gpsimd cannot touch PSUM on hw)
                            ksb = wk.tile([128, 384], FP, tag="ksb")
                            nc.scalar.copy(ksb, kps)
                            t1 = wk.tile([128, 384], FP, tag="t1")
                            nc.gpsimd.tensor_mul(t1, ksb, cosb)
                            t2 = wk.tile([128, 384], FP, tag="t2")
                            ksb3 = ksb.rearrange("p (x two) -> p x two", two=2)
                            t23 = t2.rearrange("p (x two) -> p x two", two=2)
                            sin_e = bass.AP(
                                tensor=ropa.tensor,
                                offset=ropa.offset + (b * 2 + 1) * D,
                                ap=[ropa.ap[0], [0, 6], [2, 32]],
                            )
                            sin_o = bass.AP(
                                tensor=ropa.tensor,
                                offset=ropa.offset + (b * 2 + 1) * D + 1,
                                ap=[ropa.ap[0], [0, 6], [2, 32]],
                            )
                            nc.gpsimd.tensor_mul(t23[:, :, 0], ksb3[:, :, 1], sin_e)
                            nc.gpsimd.tensor_mul(t23[:, :, 1], ksb3[:, :, 0], sin_o)
                            krin = wk.tile([128, 384], FP, tag="krin")
                            nc.gpsimd.tensor_add(krin, t1, t2)
                            nc.scalar.activation(
                                out=ek[:, kt * 384 : (kt + 1) * 384],
                                in_=krin,
                                func=mybir.ActivationFunctionType.Exp,
                            )
                        # --- per-half rowsum + v so half 0's kv inputs are
                        # ready before half 1's v matmuls finish ---
                        sk = sa.tile([128, H], FP, tag="sk")
                        ski = sa.tile([128, H], FP, tag="ski")
                        vnx = vnxpool.tile([128, NPAIR, 130], BF, tag="vnx")
                        last_blk = st == NST - 1 and b == NB - 1
                        args = (ek, vnx, st == 0 and b == 0, last_blk)
                        for vt in range(2):
                            hsl = slice(vt * 6, vt * 6 + 6)
                            nc.vector.reduce_sum(
                                sk[:, hsl],
                                ek[:, vt * 384 : (vt + 1) * 384].rearrange(
                                    "p (h d) -> p h d", d=D),
                                axis=mybir.AxisListType.X,
                            )
                            nc.vector.reciprocal(ski[:, hsl], sk[:, hsl])
                            nc.vector.tensor_copy(
                                vnx[:, 3 * vt : 3 * vt + 3, 128:130],
                                ski[:, hsl].rearrange("p (a b) -> p a b", b=2),
                            )
                            vps = kvop.tile([128, 384], FP, tag="kv_mm")
                            for ct in range(NCT):
                                _mm(
                                    nc, vps,
                                    xa[:, ct, b * 128 : (b + 1) * 128],
                                    wkvs[ct][:, C + vt * 384 : C + (vt + 1) * 384],
                                    ct == 0, ct == NCT - 1,
                                )
                            vps4 = vps.rearrange("p (pr q d) -> p pr q d", q=2, d=D)
                            skib = bass.AP(
                                tensor=ski.tensor,
                                offset=ski.offset + vt * 6,
                                ap=[ski.ap[0], [2, 3], [1, 2], [0, D]],
                            )
                            nc.vector.tensor_mul(
                                vnx[:, 3 * vt : 3 * vt + 3, 0:128].rearrange(
                                    "p pr (q d) -> p pr q d", d=D
                                ),
                                vps4,
                                skib,
                            )
                            if last_blk:
                                # flush this bank's kv chain immediately so
                                # its assembly can start while the other
                                # half's v path is still in flight
                                emit_kv_half(args, vt)
                        pending_kv = None if last_blk else args

                for st in range(NST):
                    tsl = slice(st * ST, (st + 1) * ST)
                    if st == 0:
                        xa, ropa, cT, sT = xa0, ropa0, cT0, sT0
                    else:
                        xa = xpool.tile([128, NCT, ST], FR, tag="xa")
                        nc.sync.dma_start(
                            out=xa, in_=xpk[st].rearrange("p (a t) -> p a t", t=ST))
                        ropa = sa.tile([128, NB, 2, D], FP, tag="ropa")
                        nc.scalar.dma_start(out=ropa, in_=rope_pk[st])
                        cT = sa.tile([128, ST], FP, tag="cT")
                        nc.scalar.dma_start(out=cT, in_=cosT2[:, tsl])
                        sT = sa.tile([128, ST], FP, tag="sT")
                        nc.scalar.dma_start(out=sT, in_=sinT2[:, tsl])
                    if st == 1:
                        for ct in range(NCT):
                            dmaeng[ct % 3].dma_start(
                                out=pwgs[ct],
                                in_=pwgT[ct * 128 : (ct + 1) * 128, :])
                        nc.sync.dma_start(
                            out=vbb,
                            in_=bass.AP(tensor=vb, offset=0,
                                        ap=[[0, 128], [1, C]]))
                        nc.sync.dma_start(
                            out=cb_sb,
                            in_=bass.AP(tensor=constb, offset=0,
                                        ap=[[0, 128], [1, C]]))
                        nc.sync.dma_start(
                            out=wsb,
                            in_=bass.AP(tensor=wsn2, offset=0,
                                        ap=[[0, 128], [1, C]]))
                    do_q(st, tsl, xa, cT, sT)
                    do_blocks(st, tsl, xa, ropa)

                if pending_kv is not None:
                    emit_kv(pending_kv)
                    pending_kv = None

                # ---- kvext / ksum-broadcast assembly (bf16), per bank so
                # bank 0's pairs unblock phase 2 while bank 1 finishes.
                # ksum broadcast: one strided ACT copy per (bank, head-half)
                # covers 3 pairs at once; ACT and DVE (unlike gpsimd) may
                # read PSUM.
                for i in range(2):
                    acc = kvac[i]
                    if vb_zero:
                        # kv quadrants are plain strided copies
                        q0 = acc[0:64, 0:1]
                        nc.vector.tensor_copy(
                            kvx2[i][0:64, :, 0:64],
                            bass.AP(tensor=q0.tensor, offset=q0.offset,
                                    ap=[q0.ap[0], [130, 3], [1, 64]]))
                        q1 = acc[64:128, 64:65]
                        nc.vector.tensor_copy(
                            kvx2[i][64:128, :, 64:128],
                            bass.AP(tensor=q1.tensor, offset=q1.offset,
                                    ap=[q1.ap[0], [130, 3], [1, 64]]))
                    else:
                        for j in range(3):
                            pr = 3 * i + j
                            off = j * 130
                            h0, h1 = 2 * pr, 2 * pr + 1
                            nc.vector.scalar_tensor_tensor(
                                out=kvx2[i][0:64, j, 0:64],
                                in0=vbb[0:64, h0 * D : (h0 + 1) * D],
                                scalar=acc[0:64, off + 128 : off + 129],
                                in1=acc[0:64, off + 0 : off + 64],
                                op0=mybir.AluOpType.mult,
                                op1=mybir.AluOpType.add,
                            )
                            nc.vector.scalar_tensor_tensor(
                                out=kvx2[i][64:128, j, 64:128],
                                in0=vbb[64:128, h1 * D : (h1 + 1) * D],
                                scalar=acc[64:128, off + 129 : off + 130],
                                in1=acc[64:128, off + 64 : off + 128],
                                op0=mybir.AluOpType.mult,
                                op1=mybir.AluOpType.add,
                            )
                    b0 = acc[0:64, 128:129]
                    nc.scalar.copy(
                        ksdb[i][0:64, :, 0:64],
                        bass.AP(tensor=b0.tensor, offset=b0.offset,
                                ap=[b0.ap[0], [130, 3], [0, 64]]))
                    b1 = acc[64:128, 129:130]
                    nc.scalar.copy(
                        ksdb[i][64:128, :, 64:128],
                        bass.AP(tensor=b1.tensor, offset=b1.offset,
                                ap=[b1.ap[0], [130, 3], [0, 64]]))

            # ============ PHASE 2: attn -> LN -> proj ============
            with (
                tc.tile_pool(name="attp", bufs=2) as attpool,
                tc.tile_pool(name="o2p", bufs=6) as o2pool,
                tc.tile_pool(name="zrp", bufs=2) as zrpool,
                tc.tile_pool(name="rows", bufs=2) as rows,
                tc.tile_pool(name="colsb", bufs=4) as colsb,
                tc.tile_pool(name="w2p", bufs=2) as w2pool,
                tc.tile_pool(name="otp", bufs=4) as opool,
                tc.tile_pool(name="yps", bufs=2, space="PSUM") as ypool,
                tc.tile_pool(name="zps", bufs=2, space="PSUM") as zpool,
                tc.tile_pool(name="pps", bufs=3, space="PSUM") as ppool,
                tc.tile_pool(name="smallps", bufs=1, space="PSUM") as smallp,
            ):
                rsc = float(C) ** -0.5
                for st in range(NST):
                    tsl = slice(st * ST, (st + 1) * ST)
                    att = [attpool.tile([128, ST], FR, tag=f"att{ct}",
                                        name=f"att{ct}")
                           for ct in range(NCT)]
                    s2args = []
                    # block 0's wide proj chain interleaves into the pair
                    # loop: its ct-k matmul only needs pair k's att, so it
                    # fills the PE slots where ya/zb wait on PSUM recycling
                    pps0_b0 = ppool.tile([128, 512], FP, tag="proj")
                    for fq in range(NPAIR):
                        ya = ypool.tile([128, ST], FP, tag="yps")
                        _mmb(nc, ya, kvx2[fq // 3][:, fq % 3, :],
                             eqs[fq][:, tsl], True, True)
                        zb = zpool.tile([128, ST], FP, tag="zps")
                        _mmb(nc, zb, ksdb[fq // 3][:, fq % 3, :],
                             eqs[fq][:, tsl], True, True)
                        if fq >= 2:
                            ct = fq - 2
                            _mm(nc, pps0_b0, att[ct][:, 0:128],
                                pwgs[ct][:, 0:512], ct == 0, False)
                        zr = zrpool.tile([128, ST], FP, tag="zr")
                        nc.vector.reciprocal_approx_fast(out=zr, in_=zb)
                        # gpsimd cannot read PSUM: stage ya through SBUF (ACT)
                        ya_sb = zrpool.tile([128, ST], FP, tag="ya_sb")
                        nc.scalar.copy(ya_sb, ya)
                        nc.gpsimd.tensor_mul(att[fq], ya_sb, zr)
                    for ct in (4, 5):
                        _mm(nc, pps0_b0, att[ct][:, 0:128],
                            pwgs[ct][:, 0:512], False, ct == NCT - 1)
                    # squares emitted after the pair loop so the ACT queue
                    # drains the ya copies first (they gate PSUM recycling);
                    # pair-sum them (Pool/DVE) so the PSUM reduction needs 3
                    # matmuls instead of 6 (matmul cost is free-size * count)
                    o2s = []
                    for fq in range(NPAIR):
                        o2 = o2pool.tile([128, ST], BF, tag="o2")
                        nc.scalar.activation(
                            out=o2, in_=att[fq].bitcast(FP),
                            func=mybir.ActivationFunctionType.Square,
                        )
                        o2s.append(o2)
                    for g in range(3):
                        gs = o2pool.tile([128, ST], BF, tag="o2g")
                        eng = nc.vector if g == 1 else nc.gpsimd
                        eng.tensor_add(gs, o2s[2 * g], o2s[2 * g + 1])
                        s2args.append(gs)

                    def proj_chain(b, jt):
                        pps = ppool.tile([128, 512], FP, tag="proj")
                        bsl = slice(b * 128, (b + 1) * 128)
                        lo, n = (0, 512) if jt == 0 else (512, 258)
                        for ct in range(NCT):
                            _mm(nc, pps[:, 0:n],
                                att[ct][:, bsl],
                                pwgs[ct][:, lo : lo + n],
                                ct == 0, ct == NCT - 1)
                        return pps

                    def block_tail(st, b, pps0, pps1, s2cs, tail=False,
                                   xswap=False):
                        # LN stats as [128,1] columns; mu rides in pps1 col
                        # 256 (pwg ones-column is 1/C), s2c already has +eps.
                        # var = s2c - mu^2 computed as sqrt(s2c + (-mu^2))
                        # with the negated square as an ACT bias column.
                        mu = pps1[:, 256:257]
                        nmsq = colsb.tile([128, 1], FP, tag="nmsq")
                        nc.vector.tensor_scalar(
                            out=nmsq, in0=mu, scalar1=-1.0, scalar2=mu,
                            op0=mybir.AluOpType.mult,
                            op1=mybir.AluOpType.mult)
                        sd = colsb.tile([128, 1], FP, tag="sd")
                        nc.scalar.activation(
                            out=sd, in_=s2cs[b],
                            func=mybir.ActivationFunctionType.Sqrt,
                            bias=nmsq)
                        rc = colsb.tile([128, 1], FP, tag="rc")
                        nc.vector.reciprocal(rc, sd)
                        # out = rc*(P + mu*(-wsum)) + const; the X ops (DVE,
                        # the only engine here that may read PSUM) read and
                        # release the proj PSUM banks early. On the very last
                        # block pps1 finishes first, so x1 goes first.
                        x0 = w2pool.tile([128, 512], FP, tag="x0")
                        x1 = w2pool.tile([128, 256], FP, tag="x1")

                        def emit_x0():
                            nc.vector.scalar_tensor_tensor(
                                out=x0, in0=wsb[:, 0:512], scalar=mu, in1=pps0,
                                op0=mybir.AluOpType.mult,
                                op1=mybir.AluOpType.add)

                        def emit_x1():
                            nc.vector.scalar_tensor_tensor(
                                out=x1, in0=wsb[:, 512:768], scalar=mu,
                                in1=pps1[:, 0:256],
                                op0=mybir.AluOpType.mult,
                                op1=mybir.AluOpType.add)

                        if xswap:
                            emit_x1()
                            emit_x0()
                        else:
                            emit_x0()
                            emit_x1()
                        # Pool supports only plain tensor-tensor ops: scale by
                        # rc via a free-broadcast AP, then add the constant
                        ot0 = opool.tile([128, 512], FP, tag="ot")
                        ot1 = opool.tile([128, 512], FP, tag="ot")
                        rcb = bass.AP(tensor=rc.tensor, offset=rc.offset,
                                      ap=[rc.ap[0], [0, 512]])
                        t0 = w2pool.tile([128, 512], FP, tag="t0")
                        nc.gpsimd.tensor_mul(t0, x0, rcb)
                        nc.gpsimd.tensor_add(ot0, t0, cb_sb[:, 0:512])
                        rcb2 = bass.AP(tensor=rc.tensor, offset=rc.offset,
                                       ap=[rc.ap[0], [0, 256]])
                        t1 = w2pool.tile([128, 256], FP, tag="t1")
                        nc.gpsimd.tensor_mul(t1, x1, rcb2)
                        nc.gpsimd.tensor_add(ot1[:, 0:256], t1,
                                             cb_sb[:, 512:768])
                        row0 = st * ST + b * 128
                        if tail:
                            # final stores split across idle queues
                            nc.scalar.dma_start(
                                out=out[row0 : row0 + 128, 0:256],
                                in_=ot0[:, 0:256])
                            nc.sync.dma_start(
                                out=out[row0 : row0 + 128, 256:512],
                                in_=ot0[:, 256:512])
                            nc.gpsimd.dma_start(
                                out=out[row0 : row0 + 128, 512:768],
                                in_=ot1[:, 0:256])
                        else:
                            nc.sync.dma_start(
                                out=out[row0 : row0 + 128, 0:512], in_=ot0)
                            nc.sync.dma_start(
                                out=out[row0 : row0 + 128, 512:768],
                                in_=ot1[:, 0:256])

                    # block0's wide chain was emitted inside the pair loop;
                    # the s2 chain follows (its inputs are ready by then)
                    s2p = smallp.tile([1, ST], FP, tag="sm", name="s2p")
                    for g in range(3):
                        _mmb(nc, s2p, ones128b, s2args[g], g == 0, g == 2)
                    s2row = rows.tile([1, ST], FP, tag="s2row")
                    nc.vector.tensor_scalar(
                        out=s2row, in0=s2p, scalar1=1.0 / C, scalar2=EPS_LN,
                        op0=mybir.AluOpType.mult, op1=mybir.AluOpType.add)
                    s2ct = smallp.tile([128, NB], FP, tag="sm", name="s2ct")
                    for b in range(NB):
                        # fp32 (not f32r): free-size-1 f32r matmuls fail the
                        # hw ISA check
                        nc.tensor.matmul(
                            s2ct[:, b : b + 1],
                            s2row[:, b * 128 : (b + 1) * 128], one11,
                            start=True, stop=True)
                    s2cs = [s2ct[:, b : b + 1] for b in range(NB)]
                    pps1_b0 = proj_chain(0, 1)
                    block_tail(st, 0, pps0_b0, pps1_b0, s2cs)
                    for b in range(1, NB):
                        tail = st == NST - 1 and b == NB - 1
                        xswap = tail
                        if xswap:
                            # tail trim: narrow chain (stats source) first so
                            # rc is ready when the wide chain stops
                            pps1 = proj_chain(b, 1)
                            pps0 = proj_chain(b, 0)
                        else:
                            pps0 = proj_chain(b, 0)
                            pps1 = proj_chain(b, 1)
                        block_tail(st, b, pps0, pps1, s2cs, tail=tail,
                                   xswap=xswap)
    nc.finalize()
    return nc


@functools.lru_cache(maxsize=2)
def _get_nc(vb_zero=True):
    return build_nc(vb_zero)


def _prep_shared(qkv_w, q_bias, v_bias, norm_g, norm_b, proj_w, proj_b, rope):
    f = np.float32
    W = np.asarray(qkv_w, f)
    wqT = np.ascontiguousarray(W[0:C].T)
    wkvT = np.ascontiguousarray(W[C:].T)

    s = np.asarray(rope, f)[:, :D]
    c = np.asarray(rope, f)[:, D:]
    cos_tm = np.ones((N, D), f)
    sin_tm = np.zeros((N, D), f)
    cos_tm[NPT:] = c
    sin_tm[NPT:, 0::2] = -s[:, 0::2]
    sin_tm[NPT:, 1::2] = s[:, 1::2]
    cosT2 = np.ascontiguousarray(np.tile(cos_tm.T, (2, 1)))
    sinT2 = np.ascontiguousarray(np.tile(sin_tm.T, (2, 1)))
    # rope_pk[st, p, b, 0/1, d] = cos/sin_tm[st*512 + b*128 + p, d]
    rope_pk = np.ascontiguousarray(
        np.stack([cos_tm, sin_tm], axis=1)           # [N, 2, D]
        .reshape(NST, NB, 128, 2, D)
        .transpose(0, 2, 1, 3, 4)                    # [NST, 128, NB, 2, D]
    )

    g = np.asarray(norm_g, f)
    bb = np.asarray(norm_b, f)
    P = np.asarray(proj_w, f)
    pwg = (P * g[None, :]).T                         # [C, C]
    pwgT = np.zeros((C, 770), f)
    pwgT[:, 0:C] = pwg
    pwgT[:, C] = 1.0 / C
    wsn2 = np.ascontiguousarray(-pwg.sum(axis=0))
    constb = np.ascontiguousarray(np.asarray(proj_b, f) + P @ bb)
    return dict(
        wqT=wqT, wkvT=wkvT, cosT2=cosT2, sinT2=sinT2, rope_pk=rope_pk,
        qb=np.ascontiguousarray(np.asarray(q_bias, f)),
        vb=np.ascontiguousarray(np.asarray(v_bias, f)),
        pwgT=np.ascontiguousarray(pwgT), wsn2=wsn2, constb=constb,
    )


def kernel(x, rope, qkv_w, q_bias, v_bias, norm_g, norm_b, proj_w, proj_b,
           num_heads, num_prefix_tokens, _trace=False):
    assert int(num_heads) == H and int(num_prefix_tokens) == NPT
    x = np.asarray(x, np.float32)
    assert x.shape == (B, N, C)
    vbz = bool(np.all(np.asarray(v_bias) == 0.0))
    shared = _prep_shared(qkv_w, q_bias, v_bias, norm_g, norm_b, proj_w,
                          proj_b, rope)
    in_maps = []
    for bi in range(B):
        m = dict(shared)
        xt = x[bi].T  # [C, N]
        m["xpk"] = np.ascontiguousarray(
            xt.reshape(NCT, 128, NST, ST).transpose(2, 1, 0, 3)
            .reshape(NST, 128, NCT * ST)
        )
        in_maps.append(m)
    nc = _get_nc(vbz)
    res = run_bass_kernel_spmd(nc, in_maps, core_ids=list(range(B)),
                               trace=_trace)
    out = np.stack([res.results[bi]["out"] for bi in range(B)], axis=0)
    if _trace:
        kernel.last_results = res
    return out
